# revision 3
# baseline (speedup 1.0000x reference)
"""Trainium2 Bass kernel for nn_ContrastiveLoss (N=16384, D=2048, 8 cores).

Strategy
--------
x is sharded row-wise: core c owns rows [c*2048, (c+1)*2048).  On the host
each shard is transposed to [D, rows] layout and split losslessly into
bf16 hi/lo parts (x = hi + lo up to ~2^-17 relative), so the TensorEngine
can contract over D (the partition dim) at full bf16 rate with exact
products and fp32 PSUM accumulation.

Per core the device computes, accumulating over 16 d-tiles of 128:
  row0 += xi_hi . Xh      row1 += xi_lo . Xh      row2 += xi_hi . Xl
  row3 += ones  . Xh^2    (squares via DVE, fp16)
Host combines: dots = row0+row1+row2, norms2 = row3, then does the O(N)
exp/log/sum tail (16K elements) and returns the scalar loss.

Dropped terms (xi_lo.Xl, 2*Xh*Xl and Xl^2 in norms) are ~1e-4 relative on
norms / ~1e-3 absolute on dots -> ~5e-6 relative on the loss.
"""

import os
import sys

import numpy as np

for _p in ("/opt/trn_rl_repo",):
    if _p not in sys.path:
        sys.path.insert(0, _p)

import ml_dtypes

N_TOTAL = 16384
D = 2048
N_CORES = 8
ROWS = N_TOTAL // N_CORES  # rows per core
TEMP = 0.1
EPS_COS = 1e-8
EPS_DEN = 1e-6

BF16 = ml_dtypes.bfloat16

# Filled in by kernel(); lets test.py inspect profiling results.
LAST_RESULTS = None
_CACHED_NC = None


def _install_ntff_hook_shim():
    """Provide antenv.axon_hooks (absent in this image) so trace=True can
    profile via the axon PJRT .so; also stub out artifact upload."""
    import contextlib
    import ctypes
    import types

    import antenv
    from concourse import bass_utils

    bass_utils.upload_artifacts = lambda tmpdir: tmpdir

    try:
        import antenv.axon_hooks  # noqa: F401
        return
    except ImportError:
        pass

    so_path = "/opt/axon/libaxon_pjrt.so"
    hook = None
    if os.path.exists(so_path):
        lib = ctypes.CDLL(so_path)
        if hasattr(lib, "axon_start_nrt_profile"):
            lib.axon_start_nrt_profile.argtypes = [
                ctypes.POINTER(ctypes.c_int64),
                ctypes.c_size_t,
            ]
            lib.axon_start_nrt_profile.restype = ctypes.c_int64
            lib.axon_stop_nrt_profile.argtypes = [ctypes.c_char_p]
            lib.axon_stop_nrt_profile.restype = ctypes.c_int64

            @contextlib.contextmanager
            def hook(output_dir, device_ids):
                import jax

                jax.devices()
                if device_ids:
                    ids = (ctypes.c_int64 * len(device_ids))(*device_ids)
                    rc = lib.axon_start_nrt_profile(ids, len(device_ids))
                else:
                    rc = lib.axon_start_nrt_profile(None, 0)
                if rc != 0:
                    raise RuntimeError(f"axon_start_nrt_profile rc={rc}")
                try:
                    yield
                finally:
                    n = lib.axon_stop_nrt_profile(str(output_dir).encode())
                    print(f"profile: {n} file(s) written to {output_dir}")

    mod = types.ModuleType("antenv.axon_hooks")
    _state = {"hook": hook}
    mod.set_axon_ntff_profile_hook = lambda h: _state.__setitem__("hook", h)
    mod.get_axon_ntff_profile_hook = lambda: _state["hook"]
    sys.modules["antenv.axon_hooks"] = mod
    antenv.axon_hooks = mod


def build_nc(rows=ROWS, d=D):
    """Build the per-core Bass module (same program on every core)."""
    import concourse.bacc as bacc
    import concourse.tile as tile
    from concourse import mybir

    dt_tiles = d // 128
    n_chunks = rows // 512

    nc = bacc.Bacc("TRN2", target_bir_lowering=False, debug=False)

    xh = nc.dram_tensor("xh", [d, rows], mybir.dt.bfloat16, kind="ExternalInput")
    xl = nc.dram_tensor("xl", [d, rows], mybir.dt.bfloat16, kind="ExternalInput")
    # per d-tile t: cols 8t+0..3 = [xi_hi, xi_lo, 0, 0]; cols 8t+4..7 = [0, 0, xi_hi, 0]
    w = nc.dram_tensor("w", [128, 8 * dt_tiles], mybir.dt.bfloat16, kind="ExternalInput")
    out = nc.dram_tensor("out", [4, rows], mybir.dt.float32, kind="ExternalOutput")

    with tile.TileContext(nc) as tc:
        with (
            tc.tile_pool(name="xp", bufs=4) as xpool,
            tc.tile_pool(name="sqp", bufs=3) as sqpool,
            tc.tile_pool(name="wp", bufs=1) as wpool,
            tc.tile_pool(name="ps", bufs=1, space="PSUM") as pspool,
            tc.tile_pool(name="op", bufs=1) as opool,
        ):
            wt = wpool.tile([128, 8 * dt_tiles], mybir.dt.bfloat16)
            nc.sync.dma_start(out=wt, in_=w[:, :])
            # ones weight for the squares stream: [0, 0, 0, 1]
            onesw = wpool.tile([128, 4], mybir.dt.float16)
            nc.vector.memset(onesw, 0.0)
            nc.vector.memset(onesw[:, 3:4], 1.0)

            psum = pspool.tile([4, rows], mybir.dt.float32)

            for t in range(dt_tiles):
                xht = xpool.tile([128, rows], mybir.dt.bfloat16, tag="xh")
                nc.sync.dma_start(out=xht, in_=xh[128 * t : 128 * (t + 1), :])
                xlt = xpool.tile([128, rows], mybir.dt.bfloat16, tag="xl")
                nc.scalar.dma_start(out=xlt, in_=xl[128 * t : 128 * (t + 1), :])
                sq = sqpool.tile([128, rows], mybir.dt.float16, tag="sq")
                nc.vector.tensor_mul(sq, xht, xht)
                first = t == 0
                last = t == dt_tiles - 1
                for c in range(n_chunks):
                    sl = slice(512 * c, 512 * (c + 1))
                    nc.tensor.matmul(
                        psum[0:4, sl], wt[:, 8 * t : 8 * t + 4], xht[:, sl],
                        start=first, stop=False,
                    )
                    nc.tensor.matmul(
                        psum[0:4, sl], wt[:, 8 * t + 4 : 8 * t + 8], xlt[:, sl],
                        start=False, stop=False,
                    )
                    nc.tensor.matmul(
                        psum[0:4, sl], onesw, sq[:, sl],
                        start=False, stop=last,
                    )

            osb = opool.tile([4, rows], mybir.dt.float32)
            nc.vector.tensor_copy(osb, psum)
            nc.sync.dma_start(out=out[:, :], in_=osb)

    nc.finalize()
    return nc


def _split_hi_lo(a_f32):
    """Lossless-ish split: a ~= hi + lo with hi, lo bf16."""
    hi = a_f32.astype(BF16)
    lo = (a_f32 - hi.astype(np.float32)).astype(BF16)
    return hi, lo


def _build_weights(xi, d):
    dt_tiles = d // 128
    xih, xil = _split_hi_lo(xi)
    w = np.zeros((128, 8 * dt_tiles), dtype=BF16)
    for t in range(dt_tiles):
        seg = slice(128 * t, 128 * (t + 1))
        w[:, 8 * t + 0] = xih[seg]
        w[:, 8 * t + 1] = xil[seg]
        w[:, 8 * t + 6] = xih[seg]
    return w


def kernel(x, pos_pair):
    global LAST_RESULTS, _CACHED_NC

    from concourse.bass_utils import run_bass_kernel_spmd

    x = np.asarray(x, dtype=np.float32)
    pos_pair = np.asarray(pos_pair)
    i = int(pos_pair[0])
    j = int(pos_pair[1])

    xi = x[i].astype(np.float32)
    w = _build_weights(xi, D)

    in_maps = []
    for c in range(N_CORES):
        shard_t = np.ascontiguousarray(x[c * ROWS : (c + 1) * ROWS, :].T)  # [D, ROWS]
        th, tl = _split_hi_lo(shard_t)
        in_maps.append({"xh": th, "xl": tl, "w": w})

    if _CACHED_NC is None:
        _CACHED_NC = build_nc()
    nc = _CACHED_NC

    trace = bool(os.environ.get("KERNEL_TRACE"))
    if trace:
        try:
            _install_ntff_hook_shim()
        except Exception as exc:  # profiling is best-effort
            print(f"ntff hook shim failed: {exc}")
            trace = False
    try:
        res = run_bass_kernel_spmd(
            nc, in_maps, core_ids=list(range(N_CORES)), trace=trace
        )
    except Exception:
        if not trace:
            raise
        res = run_bass_kernel_spmd(
            nc, in_maps, core_ids=list(range(N_CORES)), trace=False
        )
    LAST_RESULTS = res

    dots = np.concatenate(
        [r["out"][0] + r["out"][1] + r["out"][2] for r in res.results]
    ).astype(np.float32)
    n2 = np.concatenate([r["out"][3] for r in res.results]).astype(np.float32)

    norms = np.maximum(np.sqrt(n2), np.float32(EPS_COS))
    ni = norms[i]
    cos = dots / (norms * ni)
    e = np.exp(cos / np.float32(TEMP))
    denom = e.sum(dtype=np.float32) - e[i]
    loss = -np.log(e[j] / (denom + np.float32(EPS_DEN)))
    return np.asarray(loss, dtype=np.float32).reshape(1)
